# revision 3
# baseline (speedup 1.0000x reference)
"""Trainium2 Bass kernel for nn_AttentionLayer (B=4, T=2048, C=1024, H=16).

Sharding (8 cores): core c = (batch b = c//2, head-group g = c%2).
Data parallel on batch, tensor parallel on heads: each core computes the
qkv projection for its 8 heads, causal flash-attention, and a partial
output projection (row split of w_proj). Host sums the two partials per
batch and re-transposes.

Per-core kernel (Bass/Tile, fp32r matmuls = TF32-like fp22 PE mode):
  phase A: qkv projection.  Q^T/K^T produced in [head_dim, t] layout
           (moving operand = x^T), V in natural [t, head_dim] layout
           (moving operand = w_v^T) with an appended ones column.
  phase B: causal attention per head-pair.  S^T = K^T.T @ Q^T row-tiled
           2 heads/matmul (contraction 64 x 2), exp on ACT (no
           max-subtract needed: logits are O(1)), causal mask via
           gpsimd.affine_select on diagonal tiles, O^T = [V|1].T @ P^T
           accumulated in PSUM; row 64 gives softmax denominators;
           normalize via reciprocal + gpsimd partition_broadcast.
  phase C: out^T = w_p^T.T @ y^T + bias (bias only on g=0 cores).
"""
from contextlib import ExitStack

import numpy as np

import concourse.bacc as bacc
import concourse.mybir as mybir
import concourse.tile as tile
from concourse.bass_utils import run_bass_kernel_spmd

F32 = mybir.dt.float32
F32R = mybir.dt.float32r
AF = mybir.ActivationFunctionType

B, T, C, H = 4, 2048, 1024, 16
HD = C // H          # 64
NH = H // 2          # heads per core: 8
QCOLS = NH * HD      # 512


def build(T=T, C=C, NH=NH, HD=HD, TQ=512, loop_iters=1):
    assert C % 128 == 0 and T % TQ == 0 and TQ % 128 == 0
    NP = NH // 2              # head pairs
    CT = C // 128             # contraction tiles
    NTB = T // TQ             # time blocks
    TT = T // 128             # tk tiles
    NO = C // 128             # out row tiles
    QC = NH * HD
    scale = 1.0 / (HD ** 0.5)

    nc = bacc.Bacc()
    xT = nc.declare_dram_parameter("xT", [C, T], F32R, isOutput=False)
    wqkT = nc.declare_dram_parameter("wqkT", [C, 2 * QC], F32R, isOutput=False)
    wvT = nc.declare_dram_parameter("wvT", [C, QC], F32R, isOutput=False)
    wpT = nc.declare_dram_parameter("wpT", [QC, C], F32R, isOutput=False)
    bias = nc.declare_dram_parameter("bias", [128, NO], F32, isOutput=False)
    outT = nc.declare_dram_parameter("outT", [C, T], F32, isOutput=True)

    with tile.TileContext(nc) as tc, ExitStack() as ctx:
        # long-lived pools first (stack allocator)
        qt_pool = ctx.enter_context(tc.tile_pool(name="qt", bufs=NP * NTB))
        kt_pool = ctx.enter_context(tc.tile_pool(name="kt", bufs=NP * NTB))
        v_pool = ctx.enter_context(tc.tile_pool(name="v", bufs=TT))
        y_pool = ctx.enter_context(tc.tile_pool(name="y", bufs=NP * NTB))
        wp_pool = ctx.enter_context(tc.tile_pool(name="wp", bufs=NP))
        bias_pool = ctx.enter_context(tc.tile_pool(name="bias", bufs=1))
        mm_ps = ctx.enter_context(tc.tile_pool(name="mmps", bufs=2, space="PSUM"))

        bias_sb = bias_pool.tile([128, NO], F32, tag="bias", name="bias_sb")
        nc.sync.dma_start(bias_sb[:], bias[:])
        ones_sb = bias_pool.tile([128, NH], F32, tag="ones", name="ones_sb")
        nc.gpsimd.memset(ones_sb[:], 1.0)
        wp_sb = [wp_pool.tile([128, C], F32R, tag="wp", name="wp") for p in range(NP)]
        for p in range(NP):
            nc.sync.dma_start(wp_sb[p][:], wpT[128 * p:128 * (p + 1), :])

        qt = {}
        kt = {}
        vt = []
        yt = {}

        def body():
            qt.clear(); kt.clear(); vt.clear(); yt.clear()
            # ---------------- Phase A: projections ----------------
            with tc.tile_pool(name="wqk_s", bufs=2 * CT) as wqk_pool, \
                 tc.tile_pool(name="wv_s", bufs=2 * CT) as wv_pool, \
                 tc.tile_pool(name="xs", bufs=CT + 2) as xs_pool:
                for tb in range(NTB):
                    xs = [xs_pool.tile([128, TQ], F32R, tag="xs", name="xs") for _ in range(CT)]
                    for c in range(CT):
                        nc.sync.dma_start(xs[c][:], xT[128 * c:128 * (c + 1), tb * TQ:(tb + 1) * TQ])
                    # Q^T and K^T j-tiles (128 rows = one head pair)
                    for jt in range(2 * NP):
                        ps = mm_ps.tile([128, TQ], F32, tag="mm", name="mm")
                        ws = []
                        for c in range(CT):
                            w = wqk_pool.tile([128, 128], F32R, tag="wqk", name="wqk")
                            nc.sync.dma_start(w[:], wqkT[128 * c:128 * (c + 1), 128 * jt:128 * (jt + 1)])
                            ws.append(w)
                        for c in range(CT):
                            nc.tensor.matmul(ps[:], ws[c][:], xs[c][:],
                                             start=(c == 0), stop=(c == CT - 1))
                        dst = qt_pool.tile([128, TQ], F32R, tag="qt", name="qt") if jt < NP else kt_pool.tile([128, TQ], F32R, tag="kt", name="kt")
                        nc.vector.tensor_copy(dst[:], ps[:])
                        if jt < NP:
                            qt[(jt, tb)] = dst
                        else:
                            kt[(jt - NP, tb)] = dst
                    # V natural-layout tiles for this time block
                    wvs = []
                    for c in range(CT):
                        w = wv_pool.tile([128, QC], F32R, tag="wv", name="wv")
                        nc.sync.dma_start(w[:], wvT[128 * c:128 * (c + 1), :])
                        wvs.append(w)
                    for ti in range(TQ // 128):
                        tt_i = tb * (TQ // 128) + ti
                        ps = mm_ps.tile([128, QC], F32, tag="mm", name="mmv")
                        for c in range(CT):
                            nc.tensor.matmul(ps[:], xs[c][:, 128 * ti:128 * (ti + 1)], wvs[c][:],
                                             start=(c == 0), stop=(c == CT - 1))
                        vtile = v_pool.tile([128, NH * (HD + 1)], F32R, tag="v", name="v")
                        v3 = vtile[:].rearrange("p (h d) -> p h d", d=HD + 1)
                        nc.vector.tensor_copy(v3[:, :, 0:HD], ps[:].rearrange("p (h d) -> p h d", d=HD))
                        nc.vector.tensor_copy(v3[:, :, HD], ones_sb[:])
                        assert len(vt) == tt_i
                        vt.append(vtile)

            # ---------------- Phase B: causal attention ----------------
            with tc.tile_pool(name="st", bufs=2, space="PSUM") as st_pool, \
                 tc.tile_pool(name="ops", bufs=2, space="PSUM") as o_ps_pool, \
                 tc.tile_pool(name="pt", bufs=3) as pt_pool, \
                 tc.tile_pool(name="rc", bufs=4) as rc_pool:
                for p in range(NP):
                    h0 = 2 * p
                    h1 = 2 * p + 1
                    for qi in range(NTB):
                        tq0 = qi * TQ
                        ntk = (tq0 + TQ) // 128
                        o0 = o_ps_pool.tile([HD + 1, TQ], F32, tag="ops", name="ops")
                        o1 = o_ps_pool.tile([HD + 1, TQ], F32, tag="ops", name="ops2")
                        for tki in range(ntk):
                            tk0 = tki * 128
                            ktile = kt[(p, tk0 // TQ)]
                            koff = tk0 % TQ
                            qtile = qt[(p, qi)]
                            st = st_pool.tile([128, 2 * TQ], F32, tag="st", name="st")
                            nc.tensor.matmul(st[:, 0:TQ], ktile[0:64, koff:koff + 128], qtile[0:64, :],
                                             start=True, stop=True)
                            nc.tensor.matmul(st[:, TQ:2 * TQ], ktile[64:128, koff:koff + 128], qtile[64:128, :],
                                             start=True, stop=True)
                            pt = pt_pool.tile([128, 2 * TQ], F32R, tag="pt", name="pt")
                            nc.scalar.activation(pt[:], st[:], AF.Exp, scale=scale)
                            if tk0 >= tq0:
                                d = tk0 - tq0
                                for half in range(2):
                                    nc.gpsimd.affine_select(
                                        out=pt[:, half * TQ:half * TQ + TQ],
                                        in_=pt[:, half * TQ:half * TQ + TQ],
                                        compare_op=mybir.AluOpType.is_ge,
                                        fill=0.0, base=-d,
                                        pattern=[[1, TQ]], channel_multiplier=-1)
                            vtile = vt[tki]
                            v3 = vtile[:].rearrange("p (h d) -> p h d", d=HD + 1)
                            nc.tensor.matmul(o0[:], v3[:, h0, :], pt[:, 0:TQ],
                                             start=(tki == 0), stop=(tki == ntk - 1))
                            nc.tensor.matmul(o1[:], v3[:, h1, :], pt[:, TQ:2 * TQ],
                                             start=(tki == 0), stop=(tki == ntk - 1))
                        ytile = y_pool.tile([128, TQ], F32R, tag="y", name="y")
                        yt[(p, qi)] = ytile
                        for h, ops in ((0, o0), (1, o1)):
                            rc = rc_pool.tile([1, TQ], F32, tag="rc", name="rc")
                            nc.vector.reciprocal(rc[:], ops[HD:HD + 1, :])
                            bc = rc_pool.tile([HD, TQ], F32, tag="bc", name="bc")
                            nc.gpsimd.partition_broadcast(bc[:], rc[:])
                            nc.vector.tensor_mul(ytile[64 * h:64 * h + 64, :], ops[0:HD, :], bc[:])

            # ---------------- Phase C: output projection ----------------
            with tc.tile_pool(name="osb", bufs=3) as osb_pool:
                for ot in range(NO):
                    for tb in range(NTB):
                        ps = mm_ps.tile([128, TQ], F32, tag="mm", name="mmo")
                        for p in range(NP):
                            nc.tensor.matmul(ps[:], wp_sb[p][:, 128 * ot:128 * (ot + 1)], yt[(p, tb)][:],
                                             start=(p == 0), stop=(p == NP - 1))
                        osb = osb_pool.tile([128, TQ], F32, tag="osb", name="osb")
                        nc.vector.tensor_scalar_add(osb[:], ps[:], bias_sb[:, ot:ot + 1])
                        nc.sync.dma_start(outT[128 * ot:128 * (ot + 1), tb * TQ:(tb + 1) * TQ], osb[:])

        if loop_iters == 1:
            body()
        else:
            with tc.For_i(0, loop_iters, 1):
                body()
    nc.finalize()
    return nc


def shard_inputs(x, w_attn, w_proj, b_proj):
    """Returns in_maps for 8 cores: core c = (b=c//2, g=c%2)."""
    wq, wk, wv = w_attn[0:C], w_attn[C:2 * C], w_attn[2 * C:3 * C]
    in_maps = []
    for core in range(8):
        b = core // 2
        g = core % 2
        rows = slice(g * QCOLS, (g + 1) * QCOLS)
        in_maps.append({
            "xT": np.ascontiguousarray(np.asarray(x[b]).T),
            "wqkT": np.ascontiguousarray(np.concatenate([wq[rows], wk[rows]], 0).T),
            "wvT": np.ascontiguousarray(wv[rows].T),
            "wpT": np.ascontiguousarray(w_proj[:, rows].T),
            "bias": (np.ascontiguousarray(b_proj.reshape(C // 128, 128).T)
                     if g == 0 else np.zeros((128, C // 128), np.float32)),
        })
    return in_maps


_NC_CACHE = {}


def kernel(x, w_attn, w_proj, b_proj):
    if "nc" not in _NC_CACHE:
        _NC_CACHE["nc"] = build()
    nc = _NC_CACHE["nc"]
    in_maps = shard_inputs(x, w_attn, w_proj, b_proj)
    res = run_bass_kernel_spmd(nc, in_maps, core_ids=list(range(8)))
    out = np.empty((B, T, C), np.float32)
    for b in range(B):
        out[b] = (res.results[2 * b]["outT"] + res.results[2 * b + 1]["outT"]).T
    return out


# revision 10
# speedup vs baseline: 1.3888x; 1.3888x over previous
"""Trainium2 Bass kernel for nn_AttentionLayer (B=4, T=2048, C=1024, H=16).

Sharding (8 cores): core c = (batch b = c//2, head-group g = c%2).
Data parallel on batch, tensor parallel on heads: each core computes the
qkv projection for its 8 heads, causal flash-attention, and a partial
output projection (row split of w_proj). Host sums the two partials per
batch and re-transposes.

Per-core kernel (Bass/Tile, fp32r matmuls = TF32-like fp22 PE mode):
  phase A: qkv projection.  Q^T/K^T produced in [head_dim, t] layout
           (moving operand = x^T), V in natural [t, head_dim] layout
           (moving operand = w_v^T) with an appended ones column.
  phase B: causal attention per head-pair.  S^T = K^T.T @ Q^T row-tiled
           2 heads/matmul (contraction 64 x 2), exp on ACT (no
           max-subtract needed: logits are O(1)), causal mask via
           gpsimd.affine_select on diagonal tiles, O^T = [V|1].T @ P^T
           accumulated in PSUM; row 64 gives softmax denominators;
           normalize via reciprocal + gpsimd partition_broadcast.
  phase C: out^T = w_p^T.T @ y^T + bias (bias only on g=0 cores).

All DRAM tensors are host-pre-tiled so every DMA is one contiguous block.
"""
from contextlib import ExitStack

import numpy as np

import concourse.bacc as bacc
import concourse.mybir as mybir
import concourse.tile as tile
from concourse.bass_utils import run_bass_kernel_spmd

F32 = mybir.dt.float32
F32R = mybir.dt.float32r
AF = mybir.ActivationFunctionType

B, T, C, H = 4, 2048, 1024, 16
HD = C // H          # 64
NH = H // 2          # heads per core: 8
QCOLS = NH * HD      # 512


def build(T=T, C=C, NH=NH, HD=HD, TQ=512, loop_iters=1):
    assert C % 128 == 0 and T % TQ == 0 and TQ % 128 == 0
    NP = NH // 2              # head pairs
    CT = C // 128             # contraction tiles
    NTB = T // TQ             # time blocks
    TT = T // 128             # tk tiles
    NO = C // 128             # out row tiles
    QC = NH * HD
    scale = 1.0 / (HD ** 0.5)

    nc = bacc.Bacc()
    xT = nc.declare_dram_parameter("xT", [CT, NTB, 128, TQ], F32R, isOutput=False)
    wqkT = nc.declare_dram_parameter("wqkT", [C, 2 * QC], F32R, isOutput=False)
    wvT = nc.declare_dram_parameter("wvT", [CT, 128, QC], F32R, isOutput=False)
    wpT = nc.declare_dram_parameter("wpT", [NP, 128, C], F32R, isOutput=False)
    bias = nc.declare_dram_parameter("bias", [128, NO], F32, isOutput=False)
    outT = nc.declare_dram_parameter("outT", [NO, NTB, 128, TQ], F32, isOutput=True)

    with tile.TileContext(nc) as tc, ExitStack() as ctx:
        # long-lived pools first (stack allocator)
        qt_pool = ctx.enter_context(tc.tile_pool(name="qt", bufs=NP * NTB))
        kt_pool = ctx.enter_context(tc.tile_pool(name="kt", bufs=NP * NTB))
        v_pool = ctx.enter_context(tc.tile_pool(name="v", bufs=TT))
        y_pool = ctx.enter_context(tc.tile_pool(name="y", bufs=NP * NTB))
        wp_pool = ctx.enter_context(tc.tile_pool(name="wp", bufs=NP))
        bias_pool = ctx.enter_context(tc.tile_pool(name="bias", bufs=1))

        bias_sb = bias_pool.tile([128, NO], F32, tag="bias", name="bias_sb")
        nc.sync.dma_start(bias_sb[:], bias[:])
        ones_sb = bias_pool.tile([128, NH], F32, tag="ones", name="ones_sb")
        nc.gpsimd.memset(ones_sb[:], 1.0)
        wp_sb = [wp_pool.tile([128, C], F32R, tag="wp", name="wp") for p in range(NP)]
        for p in range(NP):
            nc.sync.dma_start(wp_sb[p][:], wpT[p])

        qt = {}
        kt = {}
        vt = []
        yt = {}

        def body():
            qt.clear(); kt.clear(); vt.clear(); yt.clear()
            # ---------------- Phase A: projections ----------------
            with tc.tile_pool(name="wv_s", bufs=CT) as wv_pool, \
                 tc.tile_pool(name="wqk_s", bufs=CT + 4) as wqk_pool, \
                 tc.tile_pool(name="xs", bufs=CT + 2) as xs_pool, \
                 tc.tile_pool(name="mmA", bufs=2, space="PSUM") as mm_ps:
                wvs = []
                for c in range(CT):
                    w = wv_pool.tile([128, QC], F32R, tag="wv", name="wv")
                    nc.sync.dma_start(w[:], wvT[c])
                    wvs.append(w)
                for tb in range(NTB):
                    xs = [xs_pool.tile([128, TQ], F32R, tag="xs", name="xs") for _ in range(CT)]
                    for c in range(CT):
                        nc.sync.dma_start(xs[c][:], xT[c, tb])
                    # Q^T and K^T j-tiles (128 rows = one head pair);
                    # half 0 = Q cols, half 1 = K cols of wqkT
                    for half in range(2):
                        ws = []
                        for c in range(CT):
                            w = wqk_pool.tile([128, QC], F32R, tag="wqk", name="wqk")
                            nc.sync.dma_start(w[:], wqkT[128 * c:128 * (c + 1),
                                                         half * QC:(half + 1) * QC])
                            ws.append(w)
                        for jp in range(NP):
                            jt = half * NP + jp
                            ps = mm_ps.tile([128, TQ], F32, tag="mm", name="mm")
                            for c in range(CT):
                                nc.tensor.matmul(ps[:], ws[c][:, 128 * jp:128 * (jp + 1)], xs[c][:],
                                                 start=(c == 0), stop=(c == CT - 1))
                            dst = qt_pool.tile([128, TQ], F32R, tag="qt", name="qt") if jt < NP else kt_pool.tile([128, TQ], F32R, tag="kt", name="kt")
                            nc.vector.tensor_copy(dst[:], ps[:])
                            if jt < NP:
                                qt[(jt, tb)] = dst
                            else:
                                kt[(jt - NP, tb)] = dst
                    # V natural-layout tiles for this time block
                    for ti in range(TQ // 128):
                        tt_i = tb * (TQ // 128) + ti
                        ps = mm_ps.tile([128, QC], F32, tag="mm", name="mmv")
                        for c in range(CT):
                            nc.tensor.matmul(ps[:], xs[c][:, 128 * ti:128 * (ti + 1)], wvs[c][:],
                                             start=(c == 0), stop=(c == CT - 1))
                        vtile = v_pool.tile([128, NH * (HD + 1)], F32R, tag="v", name="v")
                        v3 = vtile[:].rearrange("p (h d) -> p h d", d=HD + 1)
                        nc.vector.tensor_copy(v3[:, :, 0:HD], ps[:].rearrange("p (h d) -> p h d", d=HD))
                        nc.vector.tensor_copy(v3[:, :, HD], ones_sb[:])
                        assert len(vt) == tt_i
                        vt.append(vtile)

            # ---------------- Phase B: causal attention ----------------
            # qi-outer so phase C (grouped by time block) can start early and
            # the normalize of pair p overlaps the S/exp/AV of pair p+1.
            with tc.tile_pool(name="st", bufs=2, space="PSUM") as st_pool, \
                 tc.tile_pool(name="ops", bufs=4, space="PSUM") as o_ps_pool, \
                 tc.tile_pool(name="pt", bufs=4) as pt_pool, \
                 tc.tile_pool(name="rc", bufs=4) as rc_pool, \
                 tc.tile_pool(name="osb", bufs=3) as osb_pool:
                for qi in range(NTB):
                    tq0 = qi * TQ
                    ntk = (tq0 + TQ) // 128
                    for p in range(NP):
                        h0 = 2 * p
                        h1 = 2 * p + 1
                        o0 = o_ps_pool.tile([HD + 1, TQ], F32, tag="ops", name="ops")
                        o1 = o_ps_pool.tile([HD + 1, TQ], F32, tag="ops", name="ops2")
                        for tki in range(ntk):
                            tk0 = tki * 128
                            # diagonal narrowing: only q positions >= tk0 can
                            # attend; min width 256 (f32r matmul slows <256)
                            dlt = min(max(0, tk0 - tq0), TQ - 256)
                            w = TQ - dlt
                            diag = tk0 >= tq0
                            ktile = kt[(p, tk0 // TQ)]
                            koff = tk0 % TQ
                            qtile = qt[(p, qi)]
                            st = st_pool.tile([128, 2 * TQ], F32, tag="st", name="st")
                            nc.tensor.matmul(st[:, 0:w], ktile[0:64, koff:koff + 128],
                                             qtile[0:64, dlt:TQ], start=True, stop=True)
                            nc.tensor.matmul(st[:, TQ:TQ + w], ktile[64:128, koff:koff + 128],
                                             qtile[64:128, dlt:TQ], start=True, stop=True)
                            pt = pt_pool.tile([128, 2 * TQ], F32R, tag="pt", name="pt")
                            st_v = st[:].rearrange("p (h q) -> p h q", q=TQ)[:, :, 0:w]
                            pt_v = pt[:, 0:2 * w].rearrange("p (h q) -> p h q", h=2)
                            nc.scalar.activation(pt_v, st_v, AF.Exp, scale=scale)
                            if diag:
                                bw = min(128 + (tk0 - tq0) - dlt, w)
                                base = -((tk0 - tq0) - dlt)
                                for half in range(2):
                                    nc.gpsimd.affine_select(
                                        out=pt[:, half * w:half * w + bw],
                                        in_=pt[:, half * w:half * w + bw],
                                        compare_op=mybir.AluOpType.is_ge,
                                        fill=0.0, base=base,
                                        pattern=[[1, bw]], channel_multiplier=-1)
                            vtile = vt[tki]
                            v3 = vtile[:].rearrange("p (h d) -> p h d", d=HD + 1)
                            nc.tensor.matmul(o0[:, dlt:TQ], v3[:, h0, :], pt[:, 0:w],
                                             start=(tki == 0), stop=(tki == ntk - 1))
                            nc.tensor.matmul(o1[:, dlt:TQ], v3[:, h1, :], pt[:, w:2 * w],
                                             start=(tki == 0), stop=(tki == ntk - 1))
                        ytile = y_pool.tile([128, TQ], F32R, tag="y", name="y")
                        yt[(p, qi)] = ytile
                        for h, ops in ((0, o0), (1, o1)):
                            rc = rc_pool.tile([1, TQ], F32, tag="rc", name="rc")
                            nc.vector.reciprocal(rc[:], ops[HD:HD + 1, :])
                            bc = rc_pool.tile([HD, TQ], F32, tag="bc", name="bc")
                            nc.gpsimd.partition_broadcast(bc[:], rc[:])
                            nc.vector.tensor_mul(ytile[64 * h:64 * h + 64, :], ops[0:HD, :], bc[:])

                    # ---------------- Phase C: output projection for this tb ----------------
                    tb = qi
                    for ot in range(NO):
                        ps = o_ps_pool.tile([128, TQ], F32, tag="ops", name="mmo")
                        for p in range(NP):
                            nc.tensor.matmul(ps[:], wp_sb[p][:, 128 * ot:128 * (ot + 1)], yt[(p, tb)][:],
                                             start=(p == 0), stop=(p == NP - 1))
                        osb = osb_pool.tile([128, TQ], F32, tag="osb", name="osb")
                        nc.vector.tensor_scalar_add(osb[:], ps[:], bias_sb[:, ot:ot + 1])
                        nc.sync.dma_start(outT[ot, tb], osb[:])

        if loop_iters == 1:
            body()
        else:
            with tc.For_i(0, loop_iters, 1):
                body()
    nc.finalize()
    return nc


def _tile2d(a, pr, pc):
    """[R, S] -> [R//pr, S//pc, pr, pc] contiguous tiles."""
    R, S = a.shape
    return np.ascontiguousarray(
        a.reshape(R // pr, pr, S // pc, pc).transpose(0, 2, 1, 3))


def shard_inputs(x, w_attn, w_proj, b_proj, TQ=512):
    """Returns in_maps for 8 cores: core c = (b=c//2, g=c%2)."""
    CT = C // 128
    NP = NH // 2
    NTB = T // TQ
    wq, wk, wv = w_attn[0:C], w_attn[C:2 * C], w_attn[2 * C:3 * C]
    x = np.asarray(x)
    in_maps = []
    for core in range(8):
        b = core // 2
        g = core % 2
        rows = slice(g * QCOLS, (g + 1) * QCOLS)
        xTt = _tile2d(np.asarray(x[b]).T, 128, TQ)                       # [CT,NTB,128,TQ]
        wqkTt = np.ascontiguousarray(np.concatenate([wq[rows], wk[rows]], 0).T)  # [C,2QC]
        wvTt = np.ascontiguousarray(wv[rows].T.reshape(CT, 128, QCOLS))
        wpTt = np.ascontiguousarray(w_proj[:, rows].T.reshape(NP, 128, C))
        in_maps.append({
            "xT": xTt,
            "wqkT": wqkTt,
            "wvT": wvTt,
            "wpT": wpTt,
            "bias": (np.ascontiguousarray(b_proj.reshape(C // 128, 128).T)
                     if g == 0 else np.zeros((128, C // 128), np.float32)),
        })
    return in_maps


def unshard_output(outT_tiles_pair, TQ=512):
    """outT [NO,NTB,128,TQ] partials (2 cores) -> out [T, C]."""
    s = outT_tiles_pair[0] + outT_tiles_pair[1]
    NO, NTB = C // 128, T // TQ
    return s.transpose(0, 2, 1, 3).reshape(C, T).T


_NC_CACHE = {}


def kernel(x, w_attn, w_proj, b_proj):
    if "nc" not in _NC_CACHE:
        _NC_CACHE["nc"] = build()
    nc = _NC_CACHE["nc"]
    in_maps = shard_inputs(x, w_attn, w_proj, b_proj)
    res = run_bass_kernel_spmd(nc, in_maps, core_ids=list(range(8)))
    out = np.empty((B, T, C), np.float32)
    for b in range(B):
        out[b] = unshard_output([res.results[2 * b]["outT"],
                                 res.results[2 * b + 1]["outT"]])
    return out
